# revision 11
# baseline (speedup 1.0000x reference)
# Trainium2 Bass kernel for nn_LogitsNew (dense_mlp).
#
#   u = gelu(x @ W_proj + b_proj)                       [B, D]
#   logits = (u @ W_u)[:, None, :] + ee @ W_e           [B, N, C]
#
# Sharding: data-parallel over batch B across 8 cores (4 batches/core).
#
# All data moves as bf16 (host-cast; ~0.3% norm rel err, gate is 2e-2):
# halves HBM traffic vs fp32 to ~10MB/core. ee is transposed on the host
# into k-slice-major lhsT layout, eliminating all on-device PE transposes
# of ee. Output is stored bf16 and upcast on the host.
#
# Structure (per core), ordered so the PE never starves and the tail is
# short:
#   - warmup: dummy matmuls on a memset tile so the PE HAM clock-gate
#     opens before the real work lands.
#   - phase 1: k-outer accumulation over m-tiles 0..3 (8 PSUM banks);
#     consumes eet_lo + W_e k-slice pairs as they stream in.
#   - m-tiles 4,5 k-inner (weights resident by then), plain drains.
#   - utterance path: z = x@W_proj + b (K=1 ones matmul for bias),
#     u = Gelu(z) on ACT, uT via PE transposes into ONE psum bank +
#     single copy out, y = uT.T@W_u.
#   - m-tiles 6,7 k-inner with y added in-PSUM via a K=1 ones matmul:
#     their drains are plain copies -> short tail independent of the
#     gpsimd broadcast.
#   - m0..m5 get y added in an overlapped epilogue (DVE + gpsimd split)
#     while m6/m7 matmuls run; stores stream out per-tile as bf16.
#
# DMA rings: SP (sync) carries W_e, W_proj + stores; ACT (scalar)
# carries xt/b, eet (lo then hi m-halves), W_u, and the y roundtrip.

import sys

if "/opt/trn_rl_repo" not in sys.path:
    sys.path.insert(0, "/opt/trn_rl_repo")

import numpy as np
import ml_dtypes

import concourse.bass as bass
import concourse.mybir as mybir
import concourse.tile as tile
from concourse import bacc
from concourse.bass_utils import run_bass_kernel_spmd
from concourse.masks import make_identity

P = 128
B, N, D, C = 32, 256, 1024, 1024
NCORES = 8
BPC = B // NCORES          # batches per core
KT = D // P                # 8 k-tiles over the contraction dim
FD = 512                   # matmul moving free dim (one PSUM bank of fp32)
NT = N // P                # 2 n-tiles per batch
MT = BPC * NT              # 8 m-tiles per core
HM = MT // 2 * P           # m-column split point (512)

F32 = mybir.dt.float32
BF16 = mybir.dt.bfloat16
GELU = mybir.ActivationFunctionType.Gelu
BF = ml_dtypes.bfloat16

_CACHE = {}


def _build():
    if "nc" in _CACHE:
        return _CACHE["nc"]

    nc = bacc.Bacc("TRN2", target_bir_lowering=False, debug=False, num_devices=NCORES)

    # host-packed inputs (see kernel() for the packing)
    eet = nc.dram_tensor("eet", [KT, P, MT * P], BF16, kind="ExternalInput").ap()
    we = nc.dram_tensor("we", [KT, P, C], BF16, kind="ExternalInput").ap()
    wp = nc.dram_tensor("wp", [KT, P, C], BF16, kind="ExternalInput").ap()
    wu = nc.dram_tensor("wu", [KT, P, C], BF16, kind="ExternalInput").ap()
    xt = nc.dram_tensor("xt", [P, KT, BPC], BF16, kind="ExternalInput").ap()
    bp = nc.dram_tensor("bp", [1, D], BF16, kind="ExternalInput").ap()
    out = nc.dram_tensor("logits", [MT, P, C], BF16, kind="ExternalOutput").ap()

    with tile.TileContext(nc) as tc:
        with (
            tc.tile_pool(name="const", bufs=1) as cpool,
            tc.tile_pool(name="weights", bufs=1) as wpool,
            tc.tile_pool(name="o32", bufs=1) as o32pool,
            tc.tile_pool(name="obf", bufs=1) as obfpool,
            tc.tile_pool(name="mm_ps", bufs=8, space="PSUM") as mm_ps,
        ):
            # ---- load DMAs first on each ring, in consumption order ----
            xt_sb = cpool.tile([P, KT, BPC], BF16)
            nc.scalar.dma_start(xt_sb, xt)
            b_sb = cpool.tile([1, D], BF16)
            nc.scalar.dma_start(b_sb, bp)

            we_sb = wpool.tile([P, KT, C], BF16)
            eet_sb = wpool.tile([P, KT, MT * P], BF16)
            wp_sb = wpool.tile([P, KT, C], BF16)
            wu_sb = wpool.tile([P, KT, C], BF16)
            # sync/SP ring: W_e k-slice pairs, then W_proj
            for j in range(KT // 2):
                ks = slice(2 * j, 2 * j + 2)
                nc.sync.dma_start(
                    we_sb[:, ks, :], we[ks].rearrange("a p c -> p a c")
                )
            for j in range(2):
                ks = slice(4 * j, 4 * j + 4)
                nc.sync.dma_start(
                    wp_sb[:, ks, :], wp[ks].rearrange("a p c -> p a c")
                )
            # scalar/ACT ring: eet lo halves, hi halves, then W_u
            for j in range(KT // 2):
                ks = slice(2 * j, 2 * j + 2)
                nc.scalar.dma_start(
                    eet_sb[:, ks, 0:HM],
                    eet[ks, :, 0:HM].rearrange("a p m -> p a m"),
                )
            for j in range(KT // 2):
                ks = slice(2 * j, 2 * j + 2)
                nc.scalar.dma_start(
                    eet_sb[:, ks, HM:],
                    eet[ks, :, HM:].rearrange("a p m -> p a m"),
                )
            for j in range(2):
                ks = slice(4 * j, 4 * j + 4)
                nc.scalar.dma_start(
                    wu_sb[:, ks, :], wu[ks].rearrange("a p c -> p a c")
                )

            # ---- constants ----
            ones_big = cpool.tile([P, FD], BF16)
            nc.gpsimd.memset(ones_big, 1.0)
            ident_f = cpool.tile([P, P], F32)
            make_identity(nc, ident_f)

            # ---- PE warmup: dummies while the first k-slices land ----
            dummy = mm_ps.tile([P, FD], F32, tag="mm", name="dummy")
            for _ in range(16):
                nc.tensor.matmul(
                    dummy, ones_big[:, :P], ones_big, start=True, stop=True
                )

            # ---- phase 1: k-outer accumulation over m-tiles 0..3 ----
            ph1 = {}
            for m in range(4):
                for h in range(2):
                    ph1[m, h] = mm_ps.tile([P, FD], F32, tag="mm", name=f"p1_{m}_{h}")
            for ko in range(KT):
                for m in range(4):
                    ms = slice(m * P, (m + 1) * P)
                    for h in range(2):
                        nc.tensor.matmul(
                            ph1[m, h],
                            eet_sb[:, ko, ms],
                            we_sb[:, ko, h * FD : (h + 1) * FD],
                            start=(ko == 0),
                            stop=(ko == KT - 1),
                        )
            # plain drains (y not ready yet): PSUM -> f32 SBUF on DVE
            o32 = {}
            for m in range(4):
                o32[m] = o32pool.tile([P, C], F32, tag=f"f{m}", name=f"o32_{m}")
                for h in range(2):
                    nc.vector.tensor_copy(o32[m][:, h * FD : (h + 1) * FD], ph1[m, h])

            # ---- m-tiles 4,5: k-inner, plain drains ----
            def main_mms(mt, mps):
                # leaves the accumulation groups open (stop comes from the
                # y-add ones-matmul that follows)
                ms = slice(mt * P, (mt + 1) * P)
                for ko in range(KT):
                    for h in range(2):
                        nc.tensor.matmul(
                            mps[h],
                            eet_sb[:, ko, ms],
                            we_sb[:, ko, h * FD : (h + 1) * FD],
                            start=(ko == 0),
                            stop=False,
                            skip_group_check=True,
                        )

            for mt in (4, 5):
                mps = [
                    mm_ps.tile([P, FD], F32, tag="mm", name=f"p3_{mt}_{h}")
                    for h in range(2)
                ]
                # close the groups at ko==KT-1 (no y-add for these tiles)
                ms = slice(mt * P, (mt + 1) * P)
                for ko in range(KT):
                    for h in range(2):
                        nc.tensor.matmul(
                            mps[h],
                            eet_sb[:, ko, ms],
                            we_sb[:, ko, h * FD : (h + 1) * FD],
                            start=(ko == 0),
                            stop=(ko == KT - 1),
                        )
                o32[mt] = o32pool.tile([P, C], F32, tag=f"f{mt}", name=f"o32_{mt}")
                for h in range(2):
                    nc.vector.tensor_copy(
                        o32[mt][:, h * FD : (h + 1) * FD], mps[h]
                    )

            # ---- utterance path ----
            # z = x @ W_proj + b, via K=1 ones matmul for the bias
            u32 = cpool.tile([BPC, C], F32)
            zps = {}
            for h in range(2):
                cs = slice(h * FD, (h + 1) * FD)
                zp = mm_ps.tile([P, FD], F32, tag="mm", name=f"z_{h}")
                zps[h] = zp
                for ko in range(KT):
                    nc.tensor.matmul(
                        zp[:BPC], xt_sb[:, ko, :], wp_sb[:, ko, cs],
                        start=(ko == 0), stop=False,
                    )
                nc.tensor.matmul(
                    zp[:BPC], ones_big[:1, :BPC], b_sb[:1, cs],
                    start=False, stop=True,
                )
            for h in range(2):
                cs = slice(h * FD, (h + 1) * FD)
                nc.scalar.activation(u32[:, cs], zps[h][:BPC], GELU)

            # uT via PE transposes into ONE psum bank, single copy out
            uT = cpool.tile([P, KT, BPC], BF16)
            tp_all = mm_ps.tile([P, FD], F32, tag="mm", name="tp_all")
            for ko in range(KT):
                nc.tensor.transpose(
                    tp_all[:, ko * BPC : (ko + 1) * BPC],
                    u32[:BPC, ko * P : (ko + 1) * P],
                    ident_f[:BPC, :BPC],
                )
            nc.scalar.copy(
                uT.rearrange("p a b -> p (a b)"), tp_all[:, : KT * BPC]
            )

            # y = u @ W_u ; keep bf16 copy for the in-PSUM ones-matmul add
            y_sb = cpool.tile([BPC, C], F32)
            y_bf = cpool.tile([BPC, C], BF16)
            for h in range(2):
                cs = slice(h * FD, (h + 1) * FD)
                yp = mm_ps.tile([P, FD], F32, tag="mm", name=f"y_{h}")
                for ko in range(KT):
                    nc.tensor.matmul(
                        yp[:BPC], uT[:, ko, :], wu_sb[:, ko, cs],
                        start=(ko == 0), stop=(ko == KT - 1),
                    )
                nc.scalar.copy(y_bf[:, cs], yp[:BPC])
                nc.vector.tensor_copy(y_sb[:, cs], yp[:BPC])

            # m-tile -> y-partition map: host packs eeT m-columns in batch
            # order [1, 2, 3, 0], so m6/m7 use batch-partition 0 (the only
            # partition a K=1 matmul rhs may start at) and get the in-PSUM
            # y-add; m0..m5 (partitions 1..3) take the broadcast path.
            MB = (1, 1, 2, 2, 3, 3, 0, 0)

            # broadcast y across partitions for the epilogue adds
            y_row = cpool.tile([1, BPC, C], F32)
            nc.scalar.dma_start(y_row, y_sb)
            ybc = cpool.tile([P, BPC, C], F32)
            for b2 in (1, 2, 3):
                nc.gpsimd.partition_broadcast(ybc[:, b2, :], y_row[:1, b2, :])

            # ---- m-tiles 6,7: k-inner + y added in-PSUM, plain drains ----
            obf = {}
            for mt in (6, 7):
                mps = [
                    mm_ps.tile([P, FD], F32, tag="mm", name=f"p4_{mt}_{h}")
                    for h in range(2)
                ]
                main_mms(mt, mps)
                for h in range(2):
                    cs = slice(h * FD, (h + 1) * FD)
                    nc.tensor.matmul(
                        mps[h], ones_big[:1, :P], y_bf[0:1, cs],
                        start=False, stop=True, skip_group_check=True,
                    )
                o = obfpool.tile([P, C], BF16, tag=f"o{mt}", name=f"obf_{mt}")
                obf[mt] = o
                for h in range(2):
                    nc.vector.tensor_copy(o[:, h * FD : (h + 1) * FD], mps[h])
                nc.sync.dma_start(out[mt], o)

            # ---- epilogue: add y to m0..m5, stores stream out ----
            # DVE handles m0, m2, m4; gpsimd (done broadcasting) m1, m3, m5
            for mt, eng in ((0, "v"), (1, "g"), (2, "v"), (3, "g"),
                            (4, "v"), (5, "g")):
                o = obfpool.tile([P, C], BF16, tag=f"o{mt}", name=f"obf_{mt}")
                obf[mt] = o
                engine = nc.vector if eng == "v" else nc.gpsimd
                engine.tensor_add(o, o32[mt], ybc[:, MB[mt], :])
                nc.sync.dma_start(out[mt], o)

    nc.compile()
    _CACHE["nc"] = nc
    return nc


def run(inputs, trace=False, **kwargs):
    nc = _build()
    x = np.asarray(inputs["encoded_utterance"], np.float32)
    ee = np.asarray(inputs["element_embeddings"], np.float32)
    w = np.asarray(inputs["weight_matrix"], np.float32)
    wp = np.asarray(inputs["W_proj"], np.float32)
    bp = np.asarray(inputs["b_proj"], np.float32).reshape(1, D)

    # shared weight packs (k-slice major, bf16)
    wu_p = np.ascontiguousarray(w[:D].reshape(KT, P, C)).astype(BF)
    we_p = np.ascontiguousarray(w[D:].reshape(KT, P, C)).astype(BF)
    wp_p = np.ascontiguousarray(wp.reshape(KT, P, C)).astype(BF)
    bp_p = bp.astype(BF)

    # m-tile batch permutation: m-tiles hold batches [1, 2, 3, 0] so the
    # kernel's in-PSUM y-add tiles (m6/m7) read y from partition 0
    PERM = [1, 2, 3, 0]
    INV = [3, 0, 1, 2]

    in_maps = []
    for i in range(NCORES):
        bs = slice(i * BPC, (i + 1) * BPC)
        # eeT: [4, 256, D] (batch-permuted) -> [m=1024, D] -> [KT, P, m]
        ee_c = ee[bs][PERM].reshape(BPC * N, D)
        eet_p = np.ascontiguousarray(ee_c.T.reshape(KT, P, MT * P)).astype(BF)
        # xT: [4, D] -> [D, 4] -> [KT, P, 4] -> [P, KT, 4]
        xt_p = np.ascontiguousarray(
            x[bs].T.reshape(KT, P, BPC).transpose(1, 0, 2)
        ).astype(BF)
        in_maps.append(
            {
                "eet": eet_p,
                "we": we_p,
                "wp": wp_p,
                "wu": wu_p,
                "xt": xt_p,
                "bp": bp_p,
            }
        )

    res = run_bass_kernel_spmd(
        nc, in_maps, core_ids=list(range(NCORES)), trace=trace, **kwargs
    )
    full = np.concatenate(
        [
            np.asarray(r["logits"]).astype(np.float32).reshape(BPC, N, C)[INV]
            for r in res.results
        ],
        axis=0,
    )
    return full, res


def kernel(**inputs) -> np.ndarray:
    return run(inputs, trace=False)[0]


# revision 12
# speedup vs baseline: 1.1214x; 1.1214x over previous
# Trainium2 Bass kernel for nn_LogitsNew (dense_mlp).
#
#   u = gelu(x @ W_proj + b_proj)                       [B, D]
#   logits = (u @ W_u)[:, None, :] + ee @ W_e           [B, N, C]
#
# Sharding: data-parallel over batch B across 8 cores (4 batches/core).
#
# All data moves as bf16 (host-cast; ~0.3% norm rel err, gate is 2e-2):
# halves HBM traffic vs fp32 to ~10MB/core. ee is transposed on the host
# into k-slice-major lhsT layout, eliminating all on-device PE transposes
# of ee. Output is stored bf16 and upcast on the host.
#
# Structure (per core), ordered so the PE never starves and the tail is
# short:
#   - warmup: small dummy matmuls so the PE HAM clock-gate opens early.
#   - phase 1: k-outer accumulation over m-tiles 0..3 (8 PSUM banks);
#     consumes eet_lo + W_e k-slice pairs as they stream in on two rings.
#   - m-tiles 4,5 k-inner (weights resident by then).
#   - utterance path: z = x@W_proj + b (K=1 ones matmul for bias),
#     u = Gelu(z) on ACT, uT via PE transposes into ONE psum bank +
#     single copy out, y = uT.T@W_u.
#   - m-tiles 6,7: k-inner with y added in-PSUM via a K=1 ones matmul
#     (host permutes batches so these tiles are batch-partition 0, the
#     only partition a K=1 rhs may start at) -> short store tail.
#   - m0..m5: plain bf16 stores, then y is added IN DRAM via SWDGE
#     accumulate-DMAs (gpsimd ring, FIFO-ordered after the stores),
#     fully hidden under the m6/m7 matmuls. No engine time on the adds.
#   - drains split between DVE (h0) and ACT (h1) to halve drain latency.

import sys

if "/opt/trn_rl_repo" not in sys.path:
    sys.path.insert(0, "/opt/trn_rl_repo")

import numpy as np
import ml_dtypes

import concourse.bass as bass
import concourse.mybir as mybir
import concourse.tile as tile
from concourse import bacc
from concourse.bass_utils import run_bass_kernel_spmd
from concourse.masks import make_identity

P = 128
B, N, D, C = 32, 256, 1024, 1024
NCORES = 8
BPC = B // NCORES          # batches per core
KT = D // P                # 8 k-tiles over the contraction dim
FD = 512                   # matmul moving free dim (one PSUM bank of fp32)
NT = N // P                # 2 n-tiles per batch
MT = BPC * NT              # 8 m-tiles per core
HM = MT // 2 * P           # m-column split point (512)

F32 = mybir.dt.float32
BF16 = mybir.dt.bfloat16
GELU = mybir.ActivationFunctionType.Gelu
ADD = mybir.AluOpType.add
BF = ml_dtypes.bfloat16

# m-tile -> batch-partition map: host packs eeT m-columns in batch order
# [1, 2, 3, 0], so m6/m7 are batch-partition 0 (in-PSUM y-add) and the
# rest take the DRAM-accumulate path.
MB = (1, 1, 2, 2, 3, 3, 0, 0)

_CACHE = {}


def _build():
    if "nc" in _CACHE:
        return _CACHE["nc"]

    nc = bacc.Bacc("TRN2", target_bir_lowering=False, debug=False, num_devices=NCORES)

    # host-packed inputs (see kernel() for the packing)
    eet = nc.dram_tensor("eet", [KT, P, MT * P], BF16, kind="ExternalInput").ap()
    we = nc.dram_tensor("we", [KT, P, C], BF16, kind="ExternalInput").ap()
    wp = nc.dram_tensor("wp", [KT, P, C], BF16, kind="ExternalInput").ap()
    wu = nc.dram_tensor("wu", [KT, P, C], BF16, kind="ExternalInput").ap()
    xt = nc.dram_tensor("xt", [P, KT, BPC], BF16, kind="ExternalInput").ap()
    bp = nc.dram_tensor("bp", [1, D], BF16, kind="ExternalInput").ap()
    out = nc.dram_tensor("logits", [MT, P, C], BF16, kind="ExternalOutput").ap()

    with tile.TileContext(nc) as tc:
        with (
            tc.tile_pool(name="const", bufs=1) as cpool,
            tc.tile_pool(name="weights", bufs=1) as wpool,
            tc.tile_pool(name="obf", bufs=1) as obfpool,
            tc.tile_pool(name="mm_ps", bufs=8, space="PSUM") as mm_ps,
        ):
            we_sb = wpool.tile([P, KT, C], BF16)
            eet_sb = wpool.tile([P, KT, MT * P], BF16)
            wp_sb = wpool.tile([P, KT, C], BF16)
            wu_sb = wpool.tile([P, KT, C], BF16)
            xt_sb = cpool.tile([P, KT, BPC], BF16)
            b_sb = cpool.tile([1, D], BF16)

            # sync/SP ring: W_e k-slice pairs, then W_proj
            for j in range(KT // 2):
                ks = slice(2 * j, 2 * j + 2)
                nc.sync.dma_start(
                    we_sb[:, ks, :], we[ks].rearrange("a p c -> p a c")
                )
            for j in range(2):
                ks = slice(4 * j, 4 * j + 4)
                nc.sync.dma_start(
                    wp_sb[:, ks, :], wp[ks].rearrange("a p c -> p a c")
                )
            # scalar/ACT ring: eet lo halves, hi halves, W_u, then x/b
            for j in range(2):
                ks = slice(4 * j, 4 * j + 4)
                nc.scalar.dma_start(
                    eet_sb[:, ks, 0:HM],
                    eet[ks, :, 0:HM].rearrange("a p m -> p a m"),
                )
            for j in range(2):
                ks = slice(4 * j, 4 * j + 4)
                nc.scalar.dma_start(
                    eet_sb[:, ks, HM:],
                    eet[ks, :, HM:].rearrange("a p m -> p a m"),
                )
            for j in range(2):
                ks = slice(4 * j, 4 * j + 4)
                nc.scalar.dma_start(
                    wu_sb[:, ks, :], wu[ks].rearrange("a p c -> p a c")
                )
            nc.scalar.dma_start(xt_sb, xt)
            nc.scalar.dma_start(b_sb, bp)

            # ---- constants ----
            ones_big = cpool.tile([P, FD], BF16)
            nc.gpsimd.memset(ones_big, 1.0)
            ident_f = cpool.tile([P, P], F32)
            make_identity(nc, ident_f)

            # ---- PE warmup: dummies while the first k-slices land ----
            dummy = mm_ps.tile([P, FD], F32, tag="mm", name="dummy")
            for _ in range(12):
                nc.tensor.matmul(
                    dummy[:, :256], ones_big[:, :P], ones_big[:, :256],
                    start=True, stop=True,
                )

            def drain(o, mps):
                # split the two PSUM halves across DVE and ACT
                nc.vector.tensor_copy(o[:, 0:FD], mps[0])
                nc.scalar.copy(o[:, FD:C], mps[1])

            # ---- phase 1: k-outer accumulation over m-tiles 0..3 ----
            ph1 = {}
            for m in range(4):
                for h in range(2):
                    ph1[m, h] = mm_ps.tile([P, FD], F32, tag="mm", name=f"p1_{m}_{h}")
            for ko in range(KT):
                for m in range(4):
                    ms = slice(m * P, (m + 1) * P)
                    for h in range(2):
                        nc.tensor.matmul(
                            ph1[m, h],
                            eet_sb[:, ko, ms],
                            we_sb[:, ko, h * FD : (h + 1) * FD],
                            start=(ko == 0),
                            stop=(ko == KT - 1),
                        )
            obf = {}
            for m in range(4):
                obf[m] = obfpool.tile([P, C], BF16, tag=f"o{m}", name=f"obf_{m}")
                drain(obf[m], [ph1[m, 0], ph1[m, 1]])
                nc.gpsimd.dma_start(out[m], obf[m])

            # ---- m-tiles 4,5: k-inner, plain drains + stores ----
            for mt in (4, 5):
                mps = [
                    mm_ps.tile([P, FD], F32, tag="mm", name=f"p3_{mt}_{h}")
                    for h in range(2)
                ]
                ms = slice(mt * P, (mt + 1) * P)
                for ko in range(KT):
                    for h in range(2):
                        nc.tensor.matmul(
                            mps[h],
                            eet_sb[:, ko, ms],
                            we_sb[:, ko, h * FD : (h + 1) * FD],
                            start=(ko == 0),
                            stop=(ko == KT - 1),
                        )
                obf[mt] = obfpool.tile([P, C], BF16, tag=f"o{mt}", name=f"obf_{mt}")
                drain(obf[mt], mps)
                nc.gpsimd.dma_start(out[mt], obf[mt])

            # ---- utterance path ----
            # z = x @ W_proj + b, via K=1 ones matmul for the bias
            u32 = cpool.tile([BPC, C], F32)
            zps = {}
            for h in range(2):
                cs = slice(h * FD, (h + 1) * FD)
                zp = mm_ps.tile([P, FD], F32, tag="mm", name=f"z_{h}")
                zps[h] = zp
                for ko in range(KT):
                    nc.tensor.matmul(
                        zp[:BPC], xt_sb[:, ko, :], wp_sb[:, ko, cs],
                        start=(ko == 0), stop=False,
                    )
                nc.tensor.matmul(
                    zp[:BPC], ones_big[:1, :BPC], b_sb[:1, cs],
                    start=False, stop=True,
                )
            for h in range(2):
                cs = slice(h * FD, (h + 1) * FD)
                nc.scalar.activation(u32[:, cs], zps[h][:BPC], GELU)

            # uT via PE transposes into ONE psum bank, single copy out
            uT = cpool.tile([P, KT, BPC], BF16)
            tp_all = mm_ps.tile([P, FD], F32, tag="mm", name="tp_all")
            for ko in range(KT):
                nc.tensor.transpose(
                    tp_all[:, ko * BPC : (ko + 1) * BPC],
                    u32[:BPC, ko * P : (ko + 1) * P],
                    ident_f[:BPC, :BPC],
                )
            nc.scalar.copy(
                uT.rearrange("p a b -> p (a b)"), tp_all[:, : KT * BPC]
            )

            # y = u @ W_u, kept as bf16
            y_bf = cpool.tile([BPC, C], BF16)
            for h in range(2):
                cs = slice(h * FD, (h + 1) * FD)
                yp = mm_ps.tile([P, FD], F32, tag="mm", name=f"y_{h}")
                for ko in range(KT):
                    nc.tensor.matmul(
                        yp[:BPC], uT[:, ko, :], wu_sb[:, ko, cs],
                        start=(ko == 0), stop=(ko == KT - 1),
                    )
                nc.vector.tensor_copy(y_bf[:, cs], yp[:BPC])

            # broadcast y rows (batch-partitions 1..3) for the DRAM adds
            y_row = cpool.tile([1, BPC, C], BF16)
            nc.scalar.dma_start(y_row, y_bf)
            ybc = cpool.tile([P, BPC, C], BF16)
            for b2 in (1, 2, 3):
                nc.gpsimd.partition_broadcast(ybc[:, b2, :], y_row[:1, b2, :])

            # DRAM accumulate: out[mt] += y (gpsimd ring, FIFO after the
            # plain stores above; hidden under the m6/m7 matmuls)
            for mt in range(6):
                nc.gpsimd.dma_start(out[mt], ybc[:, MB[mt], :], accum_op=ADD)

            # ---- m-tiles 6,7: k-inner + y added in-PSUM, plain drains ----
            for mt in (6, 7):
                mps = [
                    mm_ps.tile([P, FD], F32, tag="mm", name=f"p4_{mt}_{h}")
                    for h in range(2)
                ]
                ms = slice(mt * P, (mt + 1) * P)
                for ko in range(KT):
                    for h in range(2):
                        nc.tensor.matmul(
                            mps[h],
                            eet_sb[:, ko, ms],
                            we_sb[:, ko, h * FD : (h + 1) * FD],
                            start=(ko == 0),
                            stop=False,
                            skip_group_check=True,
                        )
                for h in range(2):
                    cs = slice(h * FD, (h + 1) * FD)
                    nc.tensor.matmul(
                        mps[h], ones_big[:1, :P], y_bf[0:1, cs],
                        start=False, stop=True, skip_group_check=True,
                    )
                obf[mt] = obfpool.tile([P, C], BF16, tag=f"o{mt}", name=f"obf_{mt}")
                drain(obf[mt], mps)
                nc.sync.dma_start(out[mt], obf[mt])

    nc.compile()
    _CACHE["nc"] = nc
    return nc


def run(inputs, trace=False, **kwargs):
    nc = _build()
    x = np.asarray(inputs["encoded_utterance"], np.float32)
    ee = np.asarray(inputs["element_embeddings"], np.float32)
    w = np.asarray(inputs["weight_matrix"], np.float32)
    wp = np.asarray(inputs["W_proj"], np.float32)
    bp = np.asarray(inputs["b_proj"], np.float32).reshape(1, D)

    # shared weight packs (k-slice major, bf16)
    wu_p = np.ascontiguousarray(w[:D].reshape(KT, P, C)).astype(BF)
    we_p = np.ascontiguousarray(w[D:].reshape(KT, P, C)).astype(BF)
    wp_p = np.ascontiguousarray(wp.reshape(KT, P, C)).astype(BF)
    bp_p = bp.astype(BF)

    # m-tile batch permutation (see MB above)
    PERM = [1, 2, 3, 0]
    INV = [3, 0, 1, 2]

    in_maps = []
    for i in range(NCORES):
        bs = slice(i * BPC, (i + 1) * BPC)
        # eeT: [4, 256, D] (batch-permuted) -> [m=1024, D] -> [KT, P, m]
        ee_c = ee[bs][PERM].reshape(BPC * N, D)
        eet_p = np.ascontiguousarray(ee_c.T.reshape(KT, P, MT * P)).astype(BF)
        # xT: [4, D] -> [D, 4] -> [KT, P, 4] -> [P, KT, 4]
        xt_p = np.ascontiguousarray(
            x[bs].T.reshape(KT, P, BPC).transpose(1, 0, 2)
        ).astype(BF)
        in_maps.append(
            {
                "eet": eet_p,
                "we": we_p,
                "wp": wp_p,
                "wu": wu_p,
                "xt": xt_p,
                "bp": bp_p,
            }
        )

    res = run_bass_kernel_spmd(
        nc, in_maps, core_ids=list(range(NCORES)), trace=trace, **kwargs
    )
    full = np.concatenate(
        [
            np.asarray(r["logits"]).astype(np.float32).reshape(BPC, N, C)[INV]
            for r in res.results
        ],
        axis=0,
    )
    return full, res


def kernel(**inputs) -> np.ndarray:
    return run(inputs, trace=False)[0]


# revision 17
# speedup vs baseline: 1.1472x; 1.0230x over previous
# Trainium2 Bass kernel for nn_LogitsNew (dense_mlp).
#
#   u = gelu(x @ W_proj + b_proj)                       [B, D]
#   logits = (u @ W_u)[:, None, :] + ee @ W_e           [B, N, C]
#
# Sharding: data-parallel over batch B across 8 cores (4 batches/core).
#
# All data moves as bf16 (host-cast; ~0.3% norm rel err, gate is 2e-2):
# halves HBM traffic vs fp32 to ~10MB/core. ee is transposed on the host
# into k-slice-major lhsT layout, eliminating all on-device PE transposes
# of ee. Output is stored bf16 and upcast on the host.
#
# Structure (per core), ordered so the PE never starves and the tail is
# short:
#   - warmup: small dummy matmuls so the PE HAM clock-gate opens early.
#   - phase 1: k-outer accumulation over m-tiles 0..3 (8 PSUM banks);
#     consumes eet_lo + W_e k-slice pairs as they stream in on two rings.
#   - m-tiles 4,5 k-inner; their accumulation groups STAY OPEN in PSUM
#     through the utterance phase (bank budget: 4 held + 2 z + 1 tp +
#     2 y = 7 of 8; psum tiles are allocated out of emission order so
#     the pool ring lines up).
#   - utterance path: z = x@W_proj + b (K=1 ones matmul for bias),
#     u = Gelu(z) on ACT, uT via PE transposes into ONE psum bank.
#     uT is padded to 33 columns so y lands with batch 2 at partition 0
#     and batch 3 at partition 32 -- the only partitions a K=1 matmul
#     rhs may start at.
#   - y is then added in-PSUM to m4..m7 via K=1 ones matmuls; their
#     drains are plain copies -> short, broadcast-independent tail.
#   - m0..m3 get y added in an overlapped epilogue on DVE (broadcast of
#     batch 0/1 y rows via gpsimd) while m6/m7 matmuls run.
#   - drains split between DVE (h0) and ACT (h1) to halve drain latency.

import sys

if "/opt/trn_rl_repo" not in sys.path:
    sys.path.insert(0, "/opt/trn_rl_repo")

import numpy as np
import ml_dtypes

import concourse.bass as bass
import concourse.mybir as mybir
import concourse.tile as tile
from concourse import bacc
from concourse.bass_utils import run_bass_kernel_spmd
from concourse.masks import make_identity

P = 128
B, N, D, C = 32, 256, 1024, 1024
NCORES = 8
BPC = B // NCORES          # batches per core
KT = D // P                # 8 k-tiles over the contraction dim
FD = 512                   # matmul moving free dim (one PSUM bank of fp32)
NT = N // P                # 2 n-tiles per batch
MT = BPC * NT              # 8 m-tiles per core
HM = MT // 2 * P           # m-column split point (512)
UP = 33                    # padded uT columns (batch 3 at col 32)

F32 = mybir.dt.float32
BF16 = mybir.dt.bfloat16
GELU = mybir.ActivationFunctionType.Gelu
BF = ml_dtypes.bfloat16

# batch -> y partition: b2 -> 0, b0 -> 1, b1 -> 2, b3 -> 32
YPART = {0: 1, 1: 2, 2: 0, 3: 32}

_CACHE = {}


def _build():
    if "nc" in _CACHE:
        return _CACHE["nc"]

    nc = bacc.Bacc("TRN2", target_bir_lowering=False, debug=False, num_devices=NCORES)

    # host-packed inputs (see kernel() for the packing)
    eet = nc.dram_tensor("eet", [KT, P, MT * P], BF16, kind="ExternalInput").ap()
    we = nc.dram_tensor("we", [KT, P, C], BF16, kind="ExternalInput").ap()
    wp = nc.dram_tensor("wp", [KT, P, C], BF16, kind="ExternalInput").ap()
    wu = nc.dram_tensor("wu", [KT, P, C], BF16, kind="ExternalInput").ap()
    xt = nc.dram_tensor("xt", [P, KT, BPC], BF16, kind="ExternalInput").ap()
    bp = nc.dram_tensor("bp", [1, D], BF16, kind="ExternalInput").ap()
    out = nc.dram_tensor("logits", [MT, P, C], BF16, kind="ExternalOutput").ap()

    with tile.TileContext(nc) as tc:
        with (
            tc.tile_pool(name="const", bufs=1) as cpool,
            tc.tile_pool(name="weights", bufs=1) as wpool,
            tc.tile_pool(name="o32", bufs=1) as o32pool,
            tc.tile_pool(name="obf", bufs=1) as obfpool,
            tc.tile_pool(name="mm_ps", bufs=8, space="PSUM") as mm_ps,
        ):
            we_sb = wpool.tile([P, KT, C], BF16)
            eet_sb = wpool.tile([P, KT, MT * P], BF16)
            wp_sb = wpool.tile([P, KT, C], BF16)
            wu_sb = wpool.tile([P, KT, C], BF16)
            xt_sb = cpool.tile([P, KT, BPC], BF16)
            b_sb = cpool.tile([1, D], BF16)

            # sync/SP ring: W_e k-slice pairs, then W_proj
            for j in range(KT // 2):
                ks = slice(2 * j, 2 * j + 2)
                nc.sync.dma_start(
                    we_sb[:, ks, :], we[ks].rearrange("a p c -> p a c")
                )
            for j in range(2):
                ks = slice(4 * j, 4 * j + 4)
                nc.sync.dma_start(
                    wp_sb[:, ks, :], wp[ks].rearrange("a p c -> p a c")
                )
            # scalar/ACT ring: eet lo pairs, hi pairs, W_u, then x/b
            for j in range(KT // 2):
                ks = slice(2 * j, 2 * j + 2)
                nc.scalar.dma_start(
                    eet_sb[:, ks, 0:HM],
                    eet[ks, :, 0:HM].rearrange("a p m -> p a m"),
                )
            for j in range(KT // 2):
                ks = slice(2 * j, 2 * j + 2)
                nc.scalar.dma_start(
                    eet_sb[:, ks, HM:],
                    eet[ks, :, HM:].rearrange("a p m -> p a m"),
                )
            for j in range(2):
                ks = slice(4 * j, 4 * j + 4)
                nc.scalar.dma_start(
                    wu_sb[:, ks, :], wu[ks].rearrange("a p c -> p a c")
                )
            nc.scalar.dma_start(xt_sb, xt)
            nc.scalar.dma_start(b_sb, bp)

            # ---- constants ----
            ones_big = cpool.tile([P, FD], BF16)
            nc.gpsimd.memset(ones_big, 1.0)
            ident_f = cpool.tile([P, P], F32)
            make_identity(nc, ident_f)

            # ---- PSUM allocation (out of emission order so the 8-slot
            # ring matches bank lifetimes; see header) ----
            dummy = mm_ps.tile([P, FD], F32, tag="mm", name="dummy")   # s0
            ph1 = {}
            for m in range(4):
                for h in range(2):                                     # s1-s7,s0
                    ph1[m, h] = mm_ps.tile([P, FD], F32, tag="mm", name=f"p1_{m}_{h}")
            zps = [
                mm_ps.tile([P, FD], F32, tag="mm", name=f"z_{h}")      # s1,s2
                for h in range(2)
            ]
            mp45 = {
                mt: [
                    mm_ps.tile([P, FD], F32, tag="mm", name=f"p3_{mt}_{h}")
                    for h in range(2)
                ]                                                      # s3-s6
                for mt in (4, 5)
            }
            tp_all = mm_ps.tile([P, FD], F32, tag="mm", name="tp_all")  # s7
            yps = [
                mm_ps.tile([P, FD], F32, tag="mm", name=f"y_{h}")      # s0,s1
                for h in range(2)
            ]
            mp67 = {
                mt: [
                    mm_ps.tile([P, FD], F32, tag="mm", name=f"p4_{mt}_{h}")
                    for h in range(2)
                ]                                                      # s2-s5
                for mt in (6, 7)
            }

            # ---- PE warmup: dummies while the first k-slices land ----
            for _ in range(12):
                nc.tensor.matmul(
                    dummy[:, :256], ones_big[:, :P], ones_big[:, :256],
                    start=True, stop=True,
                )

            def drain(o, mps):
                # split the two PSUM halves across DVE and ACT
                nc.vector.tensor_copy(o[:, 0:FD], mps[0])
                nc.scalar.copy(o[:, FD:C], mps[1])

            # ---- phase 1: k-outer accumulation over m-tiles 0..3 ----
            for ko in range(KT):
                for m in range(4):
                    ms = slice(m * P, (m + 1) * P)
                    for h in range(2):
                        nc.tensor.matmul(
                            ph1[m, h],
                            eet_sb[:, ko, ms],
                            we_sb[:, ko, h * FD : (h + 1) * FD],
                            start=(ko == 0),
                            stop=(ko == KT - 1),
                        )
            o32 = {}
            for m in range(4):
                o32[m] = o32pool.tile([P, C], F32, tag=f"f{m}", name=f"o32_{m}")
                drain(o32[m], [ph1[m, 0], ph1[m, 1]])

            # ---- m-tiles 4,5: k-inner, groups left OPEN for the y-add ----
            for mt in (4, 5):
                ms = slice(mt * P, (mt + 1) * P)
                for ko in range(KT):
                    for h in range(2):
                        nc.tensor.matmul(
                            mp45[mt][h],
                            eet_sb[:, ko, ms],
                            we_sb[:, ko, h * FD : (h + 1) * FD],
                            start=(ko == 0),
                            stop=False,
                            skip_group_check=True,
                        )

            # ---- utterance path ----
            # z = x @ W_proj + b, via K=1 ones matmul for the bias
            u32 = cpool.tile([BPC, C], F32)
            for h in range(2):
                cs = slice(h * FD, (h + 1) * FD)
                for ko in range(KT):
                    nc.tensor.matmul(
                        zps[h][:BPC], xt_sb[:, ko, :], wp_sb[:, ko, cs],
                        start=(ko == 0), stop=False,
                    )
                nc.tensor.matmul(
                    zps[h][:BPC], ones_big[:1, :BPC], b_sb[:1, cs],
                    start=False, stop=True,
                )
            for h in range(2):
                cs = slice(h * FD, (h + 1) * FD)
                nc.scalar.activation(u32[:, cs], zps[h][:BPC], GELU)

            # uT via PE transposes into ONE psum bank; padded copy-out so
            # y rows land at partitions {0:b2, 1:b0, 2:b1, 32:b3}
            uT = cpool.tile([P, KT, UP], BF16)
            for ko in range(KT):
                nc.tensor.transpose(
                    tp_all[:, ko * BPC : (ko + 1) * BPC],
                    u32[:BPC, ko * P : (ko + 1) * P],
                    ident_f[:BPC, :BPC],
                )
            tpv = tp_all[:, : KT * BPC].rearrange("p (a b) -> p a b", b=BPC)
            nc.scalar.copy(uT[:, :, 0], tpv[:, :, 2])
            nc.scalar.copy(uT[:, :, 1:3], tpv[:, :, 0:2])
            nc.scalar.copy(uT[:, :, 32], tpv[:, :, 3])

            # y = u @ W_u  ->  [33, C] psum; bf16 copy for the K=1 adds,
            # f32 rows 1:3 for the epilogue broadcast path
            y_bf = cpool.tile([UP, C], BF16)
            y32 = cpool.tile([3, C], F32)
            for h in range(2):
                cs = slice(h * FD, (h + 1) * FD)
                for ko in range(KT):
                    nc.tensor.matmul(
                        yps[h][:UP], uT[:, ko, :], wu_sb[:, ko, cs],
                        start=(ko == 0), stop=(ko == KT - 1),
                    )
                nc.scalar.copy(y_bf[:, cs], yps[h][:UP])
                nc.vector.tensor_copy(y32[0:3, cs], yps[h][0:3, :])

            # ---- close m4..m7 with in-PSUM y-adds, plain drains ----
            obf = {}

            def yadd_close(mt, mps):
                yp_row = YPART[mt // NT]
                for h in range(2):
                    cs = slice(h * FD, (h + 1) * FD)
                    nc.tensor.matmul(
                        mps[h],
                        ones_big[yp_row : yp_row + 1, :P],
                        y_bf[yp_row : yp_row + 1, cs],
                        start=False, stop=True, skip_group_check=True,
                    )
                o = obfpool.tile([P, C], BF16, tag=f"o{mt}", name=f"obf_{mt}")
                obf[mt] = o
                drain(o, mps)
                nc.sync.dma_start(out[mt], o)

            for mt in (4, 5):
                yadd_close(mt, mp45[mt])

            # broadcast y rows for batches 0/1 (epilogue tiles m0..m3)
            y_row = cpool.tile([1, 2, C], F32)
            nc.scalar.dma_start(y_row, y32[1:3, :])
            ybc = cpool.tile([P, 2, C], F32)
            for b2 in (0, 1):
                nc.gpsimd.partition_broadcast(ybc[:, b2, :], y_row[:1, b2, :])

            # ---- m-tiles 6,7: k-inner + y-add, interleaved epilogue ----
            def ep_add(mt):
                o = obfpool.tile([P, C], BF16, tag=f"o{mt}", name=f"obf_{mt}")
                obf[mt] = o
                nc.vector.tensor_add(o, o32[mt], ybc[:, mt // NT, :])
                nc.sync.dma_start(out[mt], o)

            for mt in (6, 7):
                ms = slice(mt * P, (mt + 1) * P)
                for ko in range(KT):
                    for h in range(2):
                        nc.tensor.matmul(
                            mp67[mt][h],
                            eet_sb[:, ko, ms],
                            we_sb[:, ko, h * FD : (h + 1) * FD],
                            start=(ko == 0),
                            stop=False,
                            skip_group_check=True,
                        )
                # epilogue adds ride the DVE while the PE crunches m6/m7
                ep_add(0 if mt == 6 else 2)
                ep_add(1 if mt == 6 else 3)
                yadd_close(mt, mp67[mt])

    nc.compile()
    _CACHE["nc"] = nc
    return nc


def run(inputs, trace=False, **kwargs):
    nc = _build()
    x = np.asarray(inputs["encoded_utterance"], np.float32)
    ee = np.asarray(inputs["element_embeddings"], np.float32)
    w = np.asarray(inputs["weight_matrix"], np.float32)
    wp = np.asarray(inputs["W_proj"], np.float32)
    bp = np.asarray(inputs["b_proj"], np.float32).reshape(1, D)

    # shared weight packs (k-slice major, bf16)
    wu_p = np.ascontiguousarray(w[:D].reshape(KT, P, C)).astype(BF)
    we_p = np.ascontiguousarray(w[D:].reshape(KT, P, C)).astype(BF)
    wp_p = np.ascontiguousarray(wp.reshape(KT, P, C)).astype(BF)
    bp_p = bp.astype(BF)

    in_maps = []
    for i in range(NCORES):
        bs = slice(i * BPC, (i + 1) * BPC)
        # eeT: [4, 256, D] -> [m=1024, D] -> [D, m] -> [KT, P, m]
        ee_c = ee[bs].reshape(BPC * N, D)
        eet_p = np.ascontiguousarray(ee_c.T.reshape(KT, P, MT * P)).astype(BF)
        # xT: [4, D] -> [D, 4] -> [KT, P, 4] -> [P, KT, 4]
        xt_p = np.ascontiguousarray(
            x[bs].T.reshape(KT, P, BPC).transpose(1, 0, 2)
        ).astype(BF)
        in_maps.append(
            {
                "eet": eet_p,
                "we": we_p,
                "wp": wp_p,
                "wu": wu_p,
                "xt": xt_p,
                "bp": bp_p,
            }
        )

    res = run_bass_kernel_spmd(
        nc, in_maps, core_ids=list(range(NCORES)), trace=trace, **kwargs
    )
    full = np.concatenate(
        [
            np.asarray(r["logits"]).astype(np.float32).reshape(BPC, N, C)
            for r in res.results
        ],
        axis=0,
    )
    return full, res


def kernel(**inputs) -> np.ndarray:
    return run(inputs, trace=False)[0]


# revision 20
# speedup vs baseline: 1.2020x; 1.0477x over previous
# Trainium2 Bass kernel for nn_LogitsNew (dense_mlp).
#
#   u = gelu(x @ W_proj + b_proj)                       [B, D]
#   logits = (u @ W_u)[:, None, :] + ee @ W_e           [B, N, C]
#
# Sharding: data-parallel over batch B across 8 cores (4 batches/core).
#
# All data moves as bf16 (host-cast; ~0.3% norm rel err, gate is 2e-2):
# halves HBM traffic vs fp32 to ~10MB/core. ee is transposed on the host
# into k-slice-major lhsT layout, eliminating all on-device PE transposes
# of ee. Output is stored bf16 and upcast on the host.
#
# Structure (per core), ordered so the PE never starves and the tail is
# short:
#   - warmup: dependency-free dummy matmuls open the PE HAM clock-gate
#     before the first k-slices land; small fillers at known sub-us
#     stall points keep it open (a >3.4us idle re-throttles to 1.2GHz).
#   - phase 1: k-outer accumulation over m-tiles 0..3 (8 PSUM banks);
#     consumes eet_lo + W_e k-slice pairs as they stream in on two rings.
#   - m-tiles 4,5 k-inner; their accumulation groups STAY OPEN in PSUM
#     through the utterance phase (bank budget: 4 held + 2 z + 1 tp +
#     2 y = 7 of 8).
#   - utterance path: z = x@W_proj + b (K=1 ones matmul for bias),
#     u = Gelu(z) on ACT, uT via PE transposes into ONE psum bank.
#     uT is padded to 33 columns so y lands with batch 2 at partition 0
#     and batch 3 at partition 32 -- the only partitions a K=1 matmul
#     rhs may start at.
#   - y is then added in-PSUM to m4..m7 via K=1 ones matmuls; their
#     drains are plain copies -> short, broadcast-independent tail.
#   - m0..m3 get y added in an overlapped epilogue on DVE (broadcast of
#     batch 0/1 y rows via gpsimd) while m6/m7 matmuls run.
#   - drains split between DVE (h0) and ACT (h1); stores split across
#     both HWDGE rings.
#
# DMA rings (per-queue ~190GB/s when both stream): sync carries W_e
# pairs -> W_u -> m4..m7 stores; ACT carries eet lo/hi -> x/b -> W_proj
# -> epilogue stores. W_u lands ~28us (y needs it ~34); W_proj ~30
# (z needs it ~29.5) -- the late z is covered by m4/m5 running first.

import sys

if "/opt/trn_rl_repo" not in sys.path:
    sys.path.insert(0, "/opt/trn_rl_repo")

import numpy as np
import ml_dtypes

import concourse.bass as bass
import concourse.mybir as mybir
import concourse.tile as tile
from concourse import bacc
from concourse.bass_utils import run_bass_kernel_spmd
from concourse.masks import make_identity

P = 128
B, N, D, C = 32, 256, 1024, 1024
NCORES = 8
BPC = B // NCORES          # batches per core
KT = D // P                # 8 k-tiles over the contraction dim
FD = 512                   # matmul moving free dim (one PSUM bank of fp32)
NT = N // P                # 2 n-tiles per batch
MT = BPC * NT              # 8 m-tiles per core
HM = MT // 2 * P           # m-column split point (512)
UP = 33                    # padded uT columns (batch 3 at col 32)

F32 = mybir.dt.float32
BF16 = mybir.dt.bfloat16
GELU = mybir.ActivationFunctionType.Gelu
BF = ml_dtypes.bfloat16

# batch -> y partition: b2 -> 0, b0 -> 1, b1 -> 2, b3 -> 32
YPART = {0: 1, 1: 2, 2: 0, 3: 32}

_CACHE = {}


def _build():
    if "nc" in _CACHE:
        return _CACHE["nc"]

    nc = bacc.Bacc("TRN2", target_bir_lowering=False, debug=False, num_devices=NCORES)

    # host-packed inputs (see kernel() for the packing)
    eet = nc.dram_tensor("eet", [KT, P, MT * P], BF16, kind="ExternalInput").ap()
    we = nc.dram_tensor("we", [KT, P, C], BF16, kind="ExternalInput").ap()
    wp = nc.dram_tensor("wp", [KT, P, C], BF16, kind="ExternalInput").ap()
    wu = nc.dram_tensor("wu", [KT, P, C], BF16, kind="ExternalInput").ap()
    xt = nc.dram_tensor("xt", [P, KT, BPC], BF16, kind="ExternalInput").ap()
    bp = nc.dram_tensor("bp", [1, D], BF16, kind="ExternalInput").ap()
    out = nc.dram_tensor("logits", [MT, P, C], BF16, kind="ExternalOutput").ap()

    with tile.TileContext(nc) as tc:
        with (
            tc.tile_pool(name="const", bufs=1) as cpool,
            tc.tile_pool(name="weights", bufs=1) as wpool,
            tc.tile_pool(name="o32", bufs=1) as o32pool,
            tc.tile_pool(name="obf", bufs=1) as obfpool,
            tc.tile_pool(name="mm_ps", bufs=8, space="PSUM") as mm_ps,
        ):
            we_sb = wpool.tile([P, KT, C], BF16)
            eet_sb = wpool.tile([P, KT, MT * P], BF16)
            wp_sb = wpool.tile([P, KT, C], BF16)
            wu_sb = wpool.tile([P, KT, C], BF16)
            xt_sb = cpool.tile([P, KT, BPC], BF16)
            b_sb = cpool.tile([1, D], BF16)

            # sync/SP ring: W_e k-slice pairs, then W_u
            for j in range(KT // 2):
                ks = slice(2 * j, 2 * j + 2)
                nc.sync.dma_start(
                    we_sb[:, ks, :], we[ks].rearrange("a p c -> p a c")
                )
            for j in range(2):
                ks = slice(4 * j, 4 * j + 4)
                nc.sync.dma_start(
                    wu_sb[:, ks, :], wu[ks].rearrange("a p c -> p a c")
                )
            # scalar/ACT ring: eet lo pairs, hi pairs, x/b, then W_proj
            for j in range(KT // 2):
                ks = slice(2 * j, 2 * j + 2)
                nc.scalar.dma_start(
                    eet_sb[:, ks, 0:HM],
                    eet[ks, :, 0:HM].rearrange("a p m -> p a m"),
                )
            for j in range(KT // 2):
                ks = slice(2 * j, 2 * j + 2)
                nc.scalar.dma_start(
                    eet_sb[:, ks, HM:],
                    eet[ks, :, HM:].rearrange("a p m -> p a m"),
                )
            nc.scalar.dma_start(xt_sb, xt)
            nc.scalar.dma_start(b_sb, bp)
            for j in range(2):
                ks = slice(4 * j, 4 * j + 4)
                nc.scalar.dma_start(
                    wp_sb[:, ks, :], wp[ks].rearrange("a p c -> p a c")
                )

            # ---- constants ----
            ones_big = cpool.tile([P, FD], BF16)
            nc.gpsimd.memset(ones_big, 1.0)
            ident_f = cpool.tile([P, P], F32)
            make_identity(nc, ident_f)

            # ---- PSUM allocation (8-slot ring; order matches bank
            # lifetimes, see header) ----
            dummy = mm_ps.tile([P, FD], F32, tag="mm", name="dummy")   # s0
            ph1 = {}
            for m in range(4):
                for h in range(2):                                     # s1-s7,s0
                    ph1[m, h] = mm_ps.tile([P, FD], F32, tag="mm", name=f"p1_{m}_{h}")
            zps = [
                mm_ps.tile([P, FD], F32, tag="mm", name=f"z_{h}")      # s1,s2
                for h in range(2)
            ]
            mp45 = {
                mt: [
                    mm_ps.tile([P, FD], F32, tag="mm", name=f"p3_{mt}_{h}")
                    for h in range(2)
                ]                                                      # s3-s6
                for mt in (4, 5)
            }
            tp_all = mm_ps.tile([P, FD], F32, tag="mm", name="tp_all")  # s7
            yps = [
                mm_ps.tile([P, FD], F32, tag="mm", name=f"y_{h}")      # s0,s1
                for h in range(2)
            ]
            mp67 = {
                mt: [
                    mm_ps.tile([P, FD], F32, tag="mm", name=f"p4_{mt}_{h}")
                    for h in range(2)
                ]                                                      # s2-s5
                for mt in (6, 7)
            }

            def filler(n):
                # HAM keep-warm: harmless matmuls on garbage during
                # known sub-us dependency stalls
                for _ in range(n):
                    nc.tensor.matmul(
                        dummy[:, :256], ones_big[:, :P], ones_big[:, :256],
                        start=True, stop=True,
                    )

            # ---- PE warmup: no input deps, starts right after preamble ----
            filler(14)

            def drain(o, mps):
                # split the two PSUM halves across DVE and ACT
                nc.vector.tensor_copy(o[:, 0:FD], mps[0])
                nc.scalar.copy(o[:, FD:C], mps[1])

            # ---- phase 1: k-outer accumulation over m-tiles 0..3 ----
            for ko in range(KT):
                for m in range(4):
                    ms = slice(m * P, (m + 1) * P)
                    for h in range(2):
                        nc.tensor.matmul(
                            ph1[m, h],
                            eet_sb[:, ko, ms],
                            we_sb[:, ko, h * FD : (h + 1) * FD],
                            start=(ko == 0),
                            stop=(ko == KT - 1),
                        )
            o32 = {}
            for m in range(4):
                o32[m] = o32pool.tile([P, C], F32, tag=f"f{m}", name=f"o32_{m}")
                drain(o32[m], [ph1[m, 0], ph1[m, 1]])

            # ---- m-tiles 4,5: k-inner, groups left OPEN for the y-add ----
            for mt in (4, 5):
                ms = slice(mt * P, (mt + 1) * P)
                for ko in range(KT):
                    for h in range(2):
                        nc.tensor.matmul(
                            mp45[mt][h],
                            eet_sb[:, ko, ms],
                            we_sb[:, ko, h * FD : (h + 1) * FD],
                            start=(ko == 0),
                            stop=False,
                            skip_group_check=True,
                        )

            # ---- utterance path ----
            filler(2)  # cover the W_proj arrival edge
            # z = x @ W_proj + b, via K=1 ones matmul for the bias
            u32 = cpool.tile([BPC, C], F32)
            for h in range(2):
                cs = slice(h * FD, (h + 1) * FD)
                for ko in range(KT):
                    nc.tensor.matmul(
                        zps[h][:BPC], xt_sb[:, ko, :], wp_sb[:, ko, cs],
                        start=(ko == 0), stop=False,
                    )
                nc.tensor.matmul(
                    zps[h][:BPC], ones_big[:1, :BPC], b_sb[:1, cs],
                    start=False, stop=True,
                )
            for h in range(2):
                cs = slice(h * FD, (h + 1) * FD)
                nc.scalar.activation(u32[:, cs], zps[h][:BPC], GELU)

            # uT via PE transposes into ONE psum bank; padded copy-out so
            # y rows land at partitions {0:b2, 1:b0, 2:b1, 32:b3}
            uT = cpool.tile([P, KT, UP], BF16)
            for ko in range(KT):
                nc.tensor.transpose(
                    tp_all[:, ko * BPC : (ko + 1) * BPC],
                    u32[:BPC, ko * P : (ko + 1) * P],
                    ident_f[:BPC, :BPC],
                )
            filler(3)  # cover the uT copy-out latency
            tpv = tp_all[:, : KT * BPC].rearrange("p (a b) -> p a b", b=BPC)
            nc.vector.tensor_copy(uT[:, :, 0], tpv[:, :, 2])
            nc.scalar.copy(uT[:, :, 1:3], tpv[:, :, 0:2])
            nc.vector.tensor_copy(uT[:, :, 32], tpv[:, :, 3])

            # y = u @ W_u  ->  [33, C] psum; bf16 copy for the K=1 adds,
            # f32 rows for the epilogue broadcast path
            y_bf = cpool.tile([UP, C], BF16)
            y32 = cpool.tile([3, C], F32)
            for h in range(2):
                cs = slice(h * FD, (h + 1) * FD)
                for ko in range(KT):
                    nc.tensor.matmul(
                        yps[h][:UP], uT[:, ko, :], wu_sb[:, ko, cs],
                        start=(ko == 0), stop=(ko == KT - 1),
                    )
                nc.scalar.copy(y_bf[:, cs], yps[h][:UP])
                nc.vector.tensor_copy(y32[0:3, cs], yps[h][0:3, :])
            filler(3)  # cover the y_bf copy latency

            # ---- close m4..m7 with in-PSUM y-adds, plain drains ----
            obf = {}

            def yadd_close(mt, mps, ring):
                yp_row = YPART[mt // NT]
                for h in range(2):
                    cs = slice(h * FD, (h + 1) * FD)
                    nc.tensor.matmul(
                        mps[h],
                        ones_big[yp_row : yp_row + 1, :P],
                        y_bf[yp_row : yp_row + 1, cs],
                        start=False, stop=True, skip_group_check=True,
                    )
                o = obfpool.tile([P, C], BF16, tag=f"o{mt}", name=f"obf_{mt}")
                obf[mt] = o
                drain(o, mps)
                ring.dma_start(out[mt], o)

            for mt in (4, 5):
                yadd_close(mt, mp45[mt], nc.sync)

            # broadcast y rows for batches 0/1 (epilogue tiles m0..m3)
            y_row = cpool.tile([1, 2, C], F32)
            nc.scalar.dma_start(y_row, y32[1:3, :])
            ybc = cpool.tile([P, 2, C], F32)
            for b2 in (0, 1):
                nc.gpsimd.partition_broadcast(ybc[:, b2, :], y_row[:1, b2, :])

            # ---- m-tiles 6,7: k-inner + y-add, interleaved epilogue ----
            def ep_add(mt):
                o = obfpool.tile([P, C], BF16, tag=f"o{mt}", name=f"obf_{mt}")
                obf[mt] = o
                nc.vector.tensor_add(o, o32[mt], ybc[:, mt // NT, :])
                nc.scalar.dma_start(out[mt], o)

            for mt in (6, 7):
                ms = slice(mt * P, (mt + 1) * P)
                for ko in range(KT):
                    for h in range(2):
                        nc.tensor.matmul(
                            mp67[mt][h],
                            eet_sb[:, ko, ms],
                            we_sb[:, ko, h * FD : (h + 1) * FD],
                            start=(ko == 0),
                            stop=False,
                            skip_group_check=True,
                        )
                # epilogue adds ride the DVE while the PE crunches m6/m7
                ep_add(0 if mt == 6 else 2)
                ep_add(1 if mt == 6 else 3)
                yadd_close(mt, mp67[mt], nc.sync)

    nc.compile()
    _CACHE["nc"] = nc
    return nc


def run(inputs, trace=False, **kwargs):
    nc = _build()
    x = np.asarray(inputs["encoded_utterance"], np.float32)
    ee = np.asarray(inputs["element_embeddings"], np.float32)
    w = np.asarray(inputs["weight_matrix"], np.float32)
    wp = np.asarray(inputs["W_proj"], np.float32)
    bp = np.asarray(inputs["b_proj"], np.float32).reshape(1, D)

    # shared weight packs (k-slice major, bf16)
    wu_p = np.ascontiguousarray(w[:D].reshape(KT, P, C)).astype(BF)
    we_p = np.ascontiguousarray(w[D:].reshape(KT, P, C)).astype(BF)
    wp_p = np.ascontiguousarray(wp.reshape(KT, P, C)).astype(BF)
    bp_p = bp.astype(BF)

    in_maps = []
    for i in range(NCORES):
        bs = slice(i * BPC, (i + 1) * BPC)
        # eeT: [4, 256, D] -> [m=1024, D] -> [D, m] -> [KT, P, m]
        ee_c = ee[bs].reshape(BPC * N, D)
        eet_p = np.ascontiguousarray(ee_c.T.reshape(KT, P, MT * P)).astype(BF)
        # xT: [4, D] -> [D, 4] -> [KT, P, 4] -> [P, KT, 4]
        xt_p = np.ascontiguousarray(
            x[bs].T.reshape(KT, P, BPC).transpose(1, 0, 2)
        ).astype(BF)
        in_maps.append(
            {
                "eet": eet_p,
                "we": we_p,
                "wp": wp_p,
                "wu": wu_p,
                "xt": xt_p,
                "bp": bp_p,
            }
        )

    res = run_bass_kernel_spmd(
        nc, in_maps, core_ids=list(range(NCORES)), trace=trace, **kwargs
    )
    full = np.concatenate(
        [
            np.asarray(r["logits"]).astype(np.float32).reshape(BPC, N, C)
            for r in res.results
        ],
        axis=0,
    )
    return full, res


def kernel(**inputs) -> np.ndarray:
    return run(inputs, trace=False)[0]


# revision 21
# speedup vs baseline: 1.2161x; 1.0118x over previous
# Trainium2 Bass kernel for nn_LogitsNew (dense_mlp).
#
#   u = gelu(x @ W_proj + b_proj)                       [B, D]
#   logits = (u @ W_u)[:, None, :] + ee @ W_e           [B, N, C]
#
# Sharding: data-parallel over batch B across 8 cores (4 batches/core).
#
# All data moves as bf16 (host-cast; ~0.3% norm rel err, gate is 2e-2):
# halves HBM traffic vs fp32 to ~10MB/core. ee is transposed on the host
# into k-slice-major lhsT layout, eliminating all on-device PE transposes
# of ee. Output is stored bf16 and upcast on the host.
#
# The kernel is PE-stream-bound (~200 N=512 matmuls at ~220ns warm), so
# the structure keeps the PE issuing back-to-back and pushes everything
# else off the critical path:
#   - warmup/filler dummies keep the HAM clock-gate at 2.4GHz (a >3.4us
#     idle re-throttles the PE to 1.2GHz).
#   - phase 1: k-outer accumulation over m-tiles 0..3 (8 PSUM banks),
#     consuming eet_lo + W_e k-slices as they stream in on two rings.
#   - utterance path SECOND (W_proj arrives ~24us): z = x@W_proj + b
#     (K=1 ones matmul bias), u = Gelu(z) on ACT, uT via PE transposes
#     into ONE psum bank, padded to 33 cols so y lands with batch 2 at
#     partition 0 and batch 3 at partition 32 (the only legal K=1 rhs
#     base partitions), y = uT.T@W_u.
#   - m-tiles 4..7 LAST, each closed by an in-PSUM K=1 y-add matmul ->
#     drains are plain copies, stores stream out, tail is one tile.
#   - m0..m3 get y added in an all-bf16 overlapped epilogue on DVE
#     (in-place, 2x DVE rate) while the PE crunches m4..m7.
#   - drains split between DVE (h0) and ACT (h1); stores split across
#     both HWDGE rings.

import sys

if "/opt/trn_rl_repo" not in sys.path:
    sys.path.insert(0, "/opt/trn_rl_repo")

import numpy as np
import ml_dtypes

import concourse.bass as bass
import concourse.mybir as mybir
import concourse.tile as tile
from concourse import bacc
from concourse.bass_utils import run_bass_kernel_spmd
from concourse.masks import make_identity

P = 128
B, N, D, C = 32, 256, 1024, 1024
NCORES = 8
BPC = B // NCORES          # batches per core
KT = D // P                # 8 k-tiles over the contraction dim
FD = 512                   # matmul moving free dim (one PSUM bank of fp32)
NT = N // P                # 2 n-tiles per batch
MT = BPC * NT              # 8 m-tiles per core
HM = MT // 2 * P           # m-column split point (512)
UP = 33                    # padded uT columns (batch 3 at col 32)

F32 = mybir.dt.float32
BF16 = mybir.dt.bfloat16
GELU = mybir.ActivationFunctionType.Gelu
BF = ml_dtypes.bfloat16

# batch -> y partition: b2 -> 0, b0 -> 1, b1 -> 2, b3 -> 32
YPART = {0: 1, 1: 2, 2: 0, 3: 32}

_CACHE = {}


def _build():
    if "nc" in _CACHE:
        return _CACHE["nc"]

    nc = bacc.Bacc("TRN2", target_bir_lowering=False, debug=False, num_devices=NCORES)

    # host-packed inputs (see kernel() for the packing)
    eet = nc.dram_tensor("eet", [KT, P, MT * P], BF16, kind="ExternalInput").ap()
    we = nc.dram_tensor("we", [KT, P, C], BF16, kind="ExternalInput").ap()
    wp = nc.dram_tensor("wp", [KT, P, C], BF16, kind="ExternalInput").ap()
    wu = nc.dram_tensor("wu", [KT, P, C], BF16, kind="ExternalInput").ap()
    xt = nc.dram_tensor("xt", [P, KT, BPC], BF16, kind="ExternalInput").ap()
    bp = nc.dram_tensor("bp", [1, D], BF16, kind="ExternalInput").ap()
    out = nc.dram_tensor("logits", [MT, P, C], BF16, kind="ExternalOutput").ap()

    with tile.TileContext(nc) as tc:
        with (
            tc.tile_pool(name="const", bufs=1) as cpool,
            tc.tile_pool(name="weights", bufs=1) as wpool,
            tc.tile_pool(name="opre", bufs=1) as oprepool,
            tc.tile_pool(name="obf", bufs=1) as obfpool,
            tc.tile_pool(name="mm_ps", bufs=8, space="PSUM") as mm_ps,
        ):
            we_sb = wpool.tile([P, KT, C], BF16)
            eet_sb = wpool.tile([P, KT, MT * P], BF16)
            wp_sb = wpool.tile([P, KT, C], BF16)
            wu_sb = wpool.tile([P, KT, C], BF16)
            xt_sb = cpool.tile([P, KT, BPC], BF16)
            b_sb = cpool.tile([1, D], BF16)

            # sync/SP ring: first eet_lo pair, W_e pairs, then W_u
            nc.sync.dma_start(
                eet_sb[:, 0:2, 0:HM], eet[0:2, :, 0:HM].rearrange("a p m -> p a m")
            )
            for j in range(KT // 2):
                ks = slice(2 * j, 2 * j + 2)
                nc.sync.dma_start(
                    we_sb[:, ks, :], we[ks].rearrange("a p c -> p a c")
                )
            for j in range(2):
                ks = slice(4 * j, 4 * j + 4)
                nc.sync.dma_start(
                    wu_sb[:, ks, :], wu[ks].rearrange("a p c -> p a c")
                )
            # scalar/ACT ring: eet_lo pairs 1-3, x/b, W_proj, eet_hi
            for j in range(1, KT // 2):
                ks = slice(2 * j, 2 * j + 2)
                nc.scalar.dma_start(
                    eet_sb[:, ks, 0:HM],
                    eet[ks, :, 0:HM].rearrange("a p m -> p a m"),
                )
            nc.scalar.dma_start(xt_sb, xt)
            nc.scalar.dma_start(b_sb, bp)
            for j in range(2):
                ks = slice(4 * j, 4 * j + 4)
                nc.scalar.dma_start(
                    wp_sb[:, ks, :], wp[ks].rearrange("a p c -> p a c")
                )
            for j in range(KT // 2):
                ks = slice(2 * j, 2 * j + 2)
                nc.scalar.dma_start(
                    eet_sb[:, ks, HM:],
                    eet[ks, :, HM:].rearrange("a p m -> p a m"),
                )

            # ---- constants ----
            ones_big = cpool.tile([P, FD], BF16)
            nc.gpsimd.memset(ones_big, 1.0)
            ident_f = cpool.tile([P, P], F32)
            make_identity(nc, ident_f)

            # ---- PSUM allocation (8-slot ring; order matches bank
            # lifetimes) ----
            dummy = mm_ps.tile([P, FD], F32, tag="mm", name="dummy")   # s0
            ph1 = {}
            for m in range(4):
                for h in range(2):                                     # s1-s7,s0
                    ph1[m, h] = mm_ps.tile([P, FD], F32, tag="mm", name=f"p1_{m}_{h}")
            zps = [
                mm_ps.tile([P, FD], F32, tag="mm", name=f"z_{h}")      # s1,s2
                for h in range(2)
            ]
            tp_all = mm_ps.tile([P, FD], F32, tag="mm", name="tp_all")  # s3
            yps = [
                mm_ps.tile([P, FD], F32, tag="mm", name=f"y_{h}")      # s4,s5
                for h in range(2)
            ]
            mp = {}
            for mt in (4, 5, 6, 7):                                    # s6,s7,s0..s5
                mp[mt] = [
                    mm_ps.tile([P, FD], F32, tag="mm", name=f"p3_{mt}_{h}")
                    for h in range(2)
                ]

            def filler(n):
                # HAM keep-warm: harmless matmuls during sub-us stalls
                for _ in range(n):
                    nc.tensor.matmul(
                        dummy[:, :256], ones_big[:, :P], ones_big[:, :256],
                        start=True, stop=True,
                    )

            # ---- PE warmup ----
            filler(14)

            def drain(o, mps):
                # split the two PSUM halves across DVE and ACT
                nc.vector.tensor_copy(o[:, 0:FD], mps[0])
                nc.scalar.copy(o[:, FD:C], mps[1])

            # ---- phase 1: k-outer accumulation over m-tiles 0..3 ----
            for ko in range(KT):
                for m in range(4):
                    ms = slice(m * P, (m + 1) * P)
                    for h in range(2):
                        nc.tensor.matmul(
                            ph1[m, h],
                            eet_sb[:, ko, ms],
                            we_sb[:, ko, h * FD : (h + 1) * FD],
                            start=(ko == 0),
                            stop=(ko == KT - 1),
                        )
            opre = {}
            for m in range(4):
                opre[m] = oprepool.tile([P, C], BF16, tag=f"f{m}", name=f"opre_{m}")
                drain(opre[m], [ph1[m, 0], ph1[m, 1]])

            # ---- utterance path ----
            filler(2)  # cover the W_proj arrival edge
            # z = x @ W_proj + b, via K=1 ones matmul for the bias
            u32 = cpool.tile([BPC, C], F32)
            for h in range(2):
                cs = slice(h * FD, (h + 1) * FD)
                for ko in range(KT):
                    nc.tensor.matmul(
                        zps[h][:BPC], xt_sb[:, ko, :], wp_sb[:, ko, cs],
                        start=(ko == 0), stop=False,
                    )
                nc.tensor.matmul(
                    zps[h][:BPC], ones_big[:1, :BPC], b_sb[:1, cs],
                    start=False, stop=True,
                )
            for h in range(2):
                cs = slice(h * FD, (h + 1) * FD)
                nc.scalar.activation(u32[:, cs], zps[h][:BPC], GELU)

            # uT via PE transposes into ONE psum bank; padded copy-out so
            # y rows land at partitions {0:b2, 1:b0, 2:b1, 32:b3}
            uT = cpool.tile([P, KT, UP], BF16)
            for ko in range(KT):
                nc.tensor.transpose(
                    tp_all[:, ko * BPC : (ko + 1) * BPC],
                    u32[:BPC, ko * P : (ko + 1) * P],
                    ident_f[:BPC, :BPC],
                )
            filler(3)  # cover the uT copy-out latency
            tpv = tp_all[:, : KT * BPC].rearrange("p (a b) -> p a b", b=BPC)
            nc.vector.tensor_copy(uT[:, :, 0], tpv[:, :, 2])
            nc.scalar.copy(uT[:, :, 1:3], tpv[:, :, 0:2])
            nc.vector.tensor_copy(uT[:, :, 32], tpv[:, :, 3])

            # y = u @ W_u  ->  [33, C] psum -> bf16
            y_bf = cpool.tile([UP, C], BF16)
            for h in range(2):
                cs = slice(h * FD, (h + 1) * FD)
                for ko in range(KT):
                    nc.tensor.matmul(
                        yps[h][:UP], uT[:, ko, :], wu_sb[:, ko, cs],
                        start=(ko == 0), stop=(ko == KT - 1),
                    )
                nc.scalar.copy(y_bf[:, cs], yps[h][:UP])
            filler(3)  # cover the y_bf copy latency

            # epilogue machinery for m0..m3 (all bf16, off critical path):
            # broadcast y rows for batches 0/1, add in-place, store on ACT
            y_row = cpool.tile([1, 2, C], BF16)
            nc.scalar.dma_start(y_row, y_bf[1:3, :])
            ybc = cpool.tile([P, 2, C], BF16)
            for b2 in (0, 1):
                nc.gpsimd.partition_broadcast(ybc[:, b2, :], y_row[:1, b2, :])
            for m in range(4):
                nc.vector.tensor_add(opre[m], opre[m], ybc[:, m // NT, :])
                nc.scalar.dma_start(out[m], opre[m])

            # ---- m-tiles 4..7: k-inner + in-PSUM y-add, plain drains ----
            obf = {}
            for mt in (4, 5, 6, 7):
                ms = slice(mt * P, (mt + 1) * P)
                for ko in range(KT):
                    for h in range(2):
                        nc.tensor.matmul(
                            mp[mt][h],
                            eet_sb[:, ko, ms],
                            we_sb[:, ko, h * FD : (h + 1) * FD],
                            start=(ko == 0),
                            stop=False,
                            skip_group_check=True,
                        )
                yp_row = YPART[mt // NT]
                for h in range(2):
                    cs = slice(h * FD, (h + 1) * FD)
                    nc.tensor.matmul(
                        mp[mt][h],
                        ones_big[yp_row : yp_row + 1, :P],
                        y_bf[yp_row : yp_row + 1, cs],
                        start=False, stop=True, skip_group_check=True,
                    )
                o = obfpool.tile([P, C], BF16, tag=f"o{mt}", name=f"obf_{mt}")
                obf[mt] = o
                drain(o, mp[mt])
                nc.sync.dma_start(out[mt], o)

    nc.compile()
    _CACHE["nc"] = nc
    return nc


def run(inputs, trace=False, **kwargs):
    nc = _build()
    x = np.asarray(inputs["encoded_utterance"], np.float32)
    ee = np.asarray(inputs["element_embeddings"], np.float32)
    w = np.asarray(inputs["weight_matrix"], np.float32)
    wp = np.asarray(inputs["W_proj"], np.float32)
    bp = np.asarray(inputs["b_proj"], np.float32).reshape(1, D)

    # shared weight packs (k-slice major, bf16)
    wu_p = np.ascontiguousarray(w[:D].reshape(KT, P, C)).astype(BF)
    we_p = np.ascontiguousarray(w[D:].reshape(KT, P, C)).astype(BF)
    wp_p = np.ascontiguousarray(wp.reshape(KT, P, C)).astype(BF)
    bp_p = bp.astype(BF)

    in_maps = []
    for i in range(NCORES):
        bs = slice(i * BPC, (i + 1) * BPC)
        # eeT: [4, 256, D] -> [m=1024, D] -> [D, m] -> [KT, P, m]
        ee_c = ee[bs].reshape(BPC * N, D)
        eet_p = np.ascontiguousarray(ee_c.T.reshape(KT, P, MT * P)).astype(BF)
        # xT: [4, D] -> [D, 4] -> [KT, P, 4] -> [P, KT, 4]
        xt_p = np.ascontiguousarray(
            x[bs].T.reshape(KT, P, BPC).transpose(1, 0, 2)
        ).astype(BF)
        in_maps.append(
            {
                "eet": eet_p,
                "we": we_p,
                "wp": wp_p,
                "wu": wu_p,
                "xt": xt_p,
                "bp": bp_p,
            }
        )

    res = run_bass_kernel_spmd(
        nc, in_maps, core_ids=list(range(NCORES)), trace=trace, **kwargs
    )
    full = np.concatenate(
        [
            np.asarray(r["logits"]).astype(np.float32).reshape(BPC, N, C)
            for r in res.results
        ],
        axis=0,
    )
    return full, res


def kernel(**inputs) -> np.ndarray:
    return run(inputs, trace=False)[0]


# revision 24
# speedup vs baseline: 1.2472x; 1.0256x over previous
# Trainium2 Bass kernel for nn_LogitsNew (dense_mlp).
#
#   u = gelu(x @ W_proj + b_proj)                       [B, D]
#   logits = (u @ W_u)[:, None, :] + ee @ W_e           [B, N, C]
#
# Sharding: data-parallel over batch B across 8 cores (4 batches/core).
#
# All data moves as bf16 (host-cast; ~0.3% norm rel err, gate is 2e-2):
# halves HBM traffic vs fp32 to ~10MB/core. ee is transposed on the host
# into k-slice-major lhsT layout, eliminating all on-device PE transposes
# of ee. Output is stored bf16 and upcast on the host.
#
# The kernel is PE-stream-bound (~200 N=512 matmuls at ~220ns warm), so
# the structure keeps the PE issuing back-to-back and pushes everything
# else off the critical path:
#   - warmup/filler dummies keep the HAM clock-gate at 2.4GHz (a >3.4us
#     idle re-throttles the PE to 1.2GHz).
#   - phase 1: k-outer accumulation over m-tiles 0..3 (8 PSUM banks),
#     consuming eet_lo + W_e k-slices as they stream in on two rings.
#   - utterance path SECOND (W_proj arrives ~24us): z = x@W_proj + b
#     (K=1 ones matmul bias), u = Gelu(z) on ACT, uT via PE transposes
#     into ONE psum bank, padded to 33 cols so y lands with batch 2 at
#     partition 0 and batch 3 at partition 32 (the only legal K=1 rhs
#     base partitions), y = uT.T@W_u.
#   - m-tiles 4..7 LAST, each closed by an in-PSUM K=1 y-add matmul ->
#     drains are plain copies, stores stream out, tail is one tile.
#   - m0..m3 get y added in an all-bf16 overlapped epilogue on DVE
#     (in-place, 2x DVE rate) while the PE crunches m4..m7.
#   - drains split between DVE (h0) and ACT (h1); stores split across
#     both HWDGE rings.

import sys

if "/opt/trn_rl_repo" not in sys.path:
    sys.path.insert(0, "/opt/trn_rl_repo")

import numpy as np
import ml_dtypes

import concourse.bass as bass
import concourse.mybir as mybir
import concourse.tile as tile
from concourse import bacc
from concourse.bass_utils import run_bass_kernel_spmd
from concourse.masks import make_identity

P = 128
B, N, D, C = 32, 256, 1024, 1024
NCORES = 8
BPC = B // NCORES          # batches per core
KT = D // P                # 8 k-tiles over the contraction dim
FD = 512                   # matmul moving free dim (one PSUM bank of fp32)
NT = N // P                # 2 n-tiles per batch
MT = BPC * NT              # 8 m-tiles per core
HM = MT // 2 * P           # m-column split point (512)
UP = 33                    # padded uT columns (batch 3 at col 32)

F32 = mybir.dt.float32
BF16 = mybir.dt.bfloat16
GELU = mybir.ActivationFunctionType.Gelu
BF = ml_dtypes.bfloat16

# batch -> y partition: b2 -> 0, b0 -> 1, b1 -> 2, b3 -> 32
YPART = {0: 1, 1: 2, 2: 0, 3: 32}

_CACHE = {}


def _build():
    if "nc" in _CACHE:
        return _CACHE["nc"]

    nc = bacc.Bacc("TRN2", target_bir_lowering=False, debug=False, num_devices=NCORES)

    # host-packed inputs (see kernel() for the packing)
    eet = nc.dram_tensor("eet", [KT, P, MT * P], BF16, kind="ExternalInput").ap()
    we = nc.dram_tensor("we", [KT, P, C], BF16, kind="ExternalInput").ap()
    wp = nc.dram_tensor("wp", [KT, P, C], BF16, kind="ExternalInput").ap()
    wu = nc.dram_tensor("wu", [KT, P, C], BF16, kind="ExternalInput").ap()
    xt = nc.dram_tensor("xt", [P, KT, BPC], BF16, kind="ExternalInput").ap()
    bp = nc.dram_tensor("bp", [1, D], BF16, kind="ExternalInput").ap()
    out = nc.dram_tensor("logits", [MT, P, C], BF16, kind="ExternalOutput").ap()

    with tile.TileContext(nc) as tc:
        with (
            tc.tile_pool(name="const", bufs=1) as cpool,
            tc.tile_pool(name="weights", bufs=1) as wpool,
            tc.tile_pool(name="opre", bufs=1) as oprepool,
            tc.tile_pool(name="obf", bufs=1) as obfpool,
            tc.tile_pool(name="mm_ps", bufs=8, space="PSUM") as mm_ps,
        ):
            we_sb = wpool.tile([P, KT, C], BF16)
            eet_sb = wpool.tile([P, KT, MT * P], BF16)
            wp_sb = wpool.tile([P, KT, C], BF16)
            wu_sb = wpool.tile([P, KT, C], BF16)
            xt_sb = cpool.tile([P, KT, BPC], BF16)
            b_sb = cpool.tile([1, D], BF16)

            # sync/SP ring: first eet_lo pair, W_e pairs, then W_u
            nc.sync.dma_start(
                eet_sb[:, 0:2, 0:HM], eet[0:2, :, 0:HM].rearrange("a p m -> p a m")
            )
            for j in range(KT // 2):
                ks = slice(2 * j, 2 * j + 2)
                nc.sync.dma_start(
                    we_sb[:, ks, :], we[ks].rearrange("a p c -> p a c")
                )
            for j in range(2):
                ks = slice(4 * j, 4 * j + 4)
                nc.sync.dma_start(
                    wu_sb[:, ks, :], wu[ks].rearrange("a p c -> p a c")
                )
            # scalar/ACT ring: eet_lo pairs 1-3, x/b, W_proj, eet_hi
            for j in range(1, KT // 2):
                ks = slice(2 * j, 2 * j + 2)
                nc.scalar.dma_start(
                    eet_sb[:, ks, 0:HM],
                    eet[ks, :, 0:HM].rearrange("a p m -> p a m"),
                )
            nc.scalar.dma_start(xt_sb, xt)
            nc.scalar.dma_start(b_sb, bp)
            for j in range(2):
                ks = slice(4 * j, 4 * j + 4)
                nc.scalar.dma_start(
                    wp_sb[:, ks, :], wp[ks].rearrange("a p c -> p a c")
                )
            for j in range(KT // 2):
                ks = slice(2 * j, 2 * j + 2)
                nc.scalar.dma_start(
                    eet_sb[:, ks, HM:],
                    eet[ks, :, HM:].rearrange("a p m -> p a m"),
                )

            # ---- constants ----
            ones_big = cpool.tile([P, FD], BF16)
            nc.gpsimd.memset(ones_big, 1.0)
            ident_f = cpool.tile([P, P], F32)
            make_identity(nc, ident_f)

            # ---- PSUM allocation (8-slot ring; order matches bank
            # lifetimes) ----
            dummy = mm_ps.tile([P, FD], F32, tag="mm", name="dummy")   # s0
            ph1 = {}
            for m in range(4):
                for h in range(2):                                     # s1-s7,s0
                    ph1[m, h] = mm_ps.tile([P, FD], F32, tag="mm", name=f"p1_{m}_{h}")
            zps = [
                mm_ps.tile([P, FD], F32, tag="mm", name=f"z_{h}")      # s1,s2
                for h in range(2)
            ]
            tp_all = mm_ps.tile([P, FD], F32, tag="mm", name="tp_all")  # s3
            yps = [
                mm_ps.tile([P, FD], F32, tag="mm", name=f"y_{h}")      # s4,s5
                for h in range(2)
            ]
            mp = {}
            for mt in (4, 5, 6, 7):                                    # s6,s7,s0..s5
                mp[mt] = [
                    mm_ps.tile([P, FD], F32, tag="mm", name=f"p3_{mt}_{h}")
                    for h in range(2)
                ]

            def filler(n):
                # HAM keep-warm: harmless matmuls during sub-us stalls
                for _ in range(n):
                    nc.tensor.matmul(
                        dummy[:, :256], ones_big[:, :P], ones_big[:, :256],
                        start=True, stop=True,
                    )

            # ---- PE warmup ----
            filler(14)

            def drain(o, mps):
                # split the two PSUM halves across DVE and ACT
                nc.vector.tensor_copy(o[:, 0:FD], mps[0])
                nc.scalar.copy(o[:, FD:C], mps[1])

            # ---- phase 1: k-outer accumulation over m-tiles 0..3 ----
            for ko in range(KT):
                for m in range(4):
                    ms = slice(m * P, (m + 1) * P)
                    for h in range(2):
                        nc.tensor.matmul(
                            ph1[m, h],
                            eet_sb[:, ko, ms],
                            we_sb[:, ko, h * FD : (h + 1) * FD],
                            start=(ko == 0),
                            stop=(ko == KT - 1),
                        )
                if ko in (0, 1, 2):
                    filler(3)  # keep HAM busy across the supply gap
            opre = {}
            for m in range(4):
                opre[m] = oprepool.tile([P, C], BF16, tag=f"f{m}", name=f"opre_{m}")
                drain(opre[m], [ph1[m, 0], ph1[m, 1]])

            # ---- utterance path ----
            filler(2)  # cover the W_proj arrival edge
            # z = x @ W_proj + b, via K=1 ones matmul for the bias
            u32 = cpool.tile([BPC, C], F32)
            for h in range(2):
                cs = slice(h * FD, (h + 1) * FD)
                for ko in range(KT):
                    nc.tensor.matmul(
                        zps[h][:BPC], xt_sb[:, ko, :], wp_sb[:, ko, cs],
                        start=(ko == 0), stop=False,
                    )
                nc.tensor.matmul(
                    zps[h][:BPC], ones_big[:1, :BPC], b_sb[:1, cs],
                    start=False, stop=True,
                )
            for h in range(2):
                cs = slice(h * FD, (h + 1) * FD)
                nc.scalar.activation(u32[:, cs], zps[h][:BPC], GELU)

            # uT via PE transposes into ONE psum bank; padded copy-out so
            # y rows land at partitions {0:b2, 1:b0, 2:b1, 32:b3}
            uT = cpool.tile([P, KT, UP], BF16)
            for ko in range(KT):
                nc.tensor.transpose(
                    tp_all[:, ko * BPC : (ko + 1) * BPC],
                    u32[:BPC, ko * P : (ko + 1) * P],
                    ident_f[:BPC, :BPC],
                )
            filler(3)  # cover the uT copy-out latency
            tpv = tp_all[:, : KT * BPC].rearrange("p (a b) -> p a b", b=BPC)
            nc.vector.tensor_copy(uT[:, :, 0], tpv[:, :, 2])
            nc.scalar.copy(uT[:, :, 1:3], tpv[:, :, 0:2])
            nc.vector.tensor_copy(uT[:, :, 32], tpv[:, :, 3])

            # y = u @ W_u  ->  [33, C] psum -> bf16
            y_bf = cpool.tile([UP, C], BF16)
            for h in range(2):
                cs = slice(h * FD, (h + 1) * FD)
                for ko in range(KT):
                    nc.tensor.matmul(
                        yps[h][:UP], uT[:, ko, :], wu_sb[:, ko, cs],
                        start=(ko == 0), stop=(ko == KT - 1),
                    )
                nc.scalar.copy(y_bf[:, cs], yps[h][:UP])
            filler(3)  # cover the y_bf copy latency

            # epilogue machinery for m0..m3 (all bf16, off critical path):
            # broadcast y rows for batches 0/1, add in-place, store on ACT
            y_row = cpool.tile([1, 2, C], BF16)
            nc.scalar.dma_start(y_row, y_bf[1:3, :])
            ybc = cpool.tile([P, 2, C], BF16)
            for b2 in (0, 1):
                nc.gpsimd.partition_broadcast(ybc[:, b2, :], y_row[:1, b2, :])
            for m in range(4):
                nc.vector.tensor_add(opre[m], opre[m], ybc[:, m // NT, :])
                nc.scalar.dma_start(out[m], opre[m])

            # ---- m-tiles 4..7: k-inner + in-PSUM y-add, plain drains.
            # h-sequential with per-half drains + stores so the h0 drain
            # and store overlap the h1 matmuls (shortens the m7 tail) ----
            obf = {}
            for mt in (4, 5, 6, 7):
                ms = slice(mt * P, (mt + 1) * P)
                yp_row = YPART[mt // NT]
                o = obfpool.tile([P, C], BF16, tag=f"o{mt}", name=f"obf_{mt}")
                obf[mt] = o
                for h in range(2):
                    cs = slice(h * FD, (h + 1) * FD)
                    for ko in range(KT):
                        nc.tensor.matmul(
                            mp[mt][h],
                            eet_sb[:, ko, ms],
                            we_sb[:, ko, cs],
                            start=(ko == 0),
                            stop=False,
                            skip_group_check=True,
                        )
                    nc.tensor.matmul(
                        mp[mt][h],
                        ones_big[yp_row : yp_row + 1, :P],
                        y_bf[yp_row : yp_row + 1, cs],
                        start=False, stop=True, skip_group_check=True,
                    )
                    if h == 0:
                        nc.vector.tensor_copy(o[:, cs], mp[mt][h])
                    else:
                        nc.scalar.copy(o[:, cs], mp[mt][h])
                    nc.sync.dma_start(out[mt, :, cs], o[:, cs])

    nc.compile()
    _CACHE["nc"] = nc
    return nc


def run(inputs, trace=False, **kwargs):
    nc = _build()
    x = np.asarray(inputs["encoded_utterance"], np.float32)
    ee = np.asarray(inputs["element_embeddings"], np.float32)
    w = np.asarray(inputs["weight_matrix"], np.float32)
    wp = np.asarray(inputs["W_proj"], np.float32)
    bp = np.asarray(inputs["b_proj"], np.float32).reshape(1, D)

    # shared weight packs (k-slice major, bf16)
    wu_p = np.ascontiguousarray(w[:D].reshape(KT, P, C)).astype(BF)
    we_p = np.ascontiguousarray(w[D:].reshape(KT, P, C)).astype(BF)
    wp_p = np.ascontiguousarray(wp.reshape(KT, P, C)).astype(BF)
    bp_p = bp.astype(BF)

    in_maps = []
    for i in range(NCORES):
        bs = slice(i * BPC, (i + 1) * BPC)
        # eeT: [4, 256, D] -> [m=1024, D] -> [D, m] -> [KT, P, m]
        ee_c = ee[bs].reshape(BPC * N, D)
        eet_p = np.ascontiguousarray(ee_c.T.reshape(KT, P, MT * P)).astype(BF)
        # xT: [4, D] -> [D, 4] -> [KT, P, 4] -> [P, KT, 4]
        xt_p = np.ascontiguousarray(
            x[bs].T.reshape(KT, P, BPC).transpose(1, 0, 2)
        ).astype(BF)
        in_maps.append(
            {
                "eet": eet_p,
                "we": we_p,
                "wp": wp_p,
                "wu": wu_p,
                "xt": xt_p,
                "bp": bp_p,
            }
        )

    res = run_bass_kernel_spmd(
        nc, in_maps, core_ids=list(range(NCORES)), trace=trace, **kwargs
    )
    full = np.concatenate(
        [
            np.asarray(r["logits"]).astype(np.float32).reshape(BPC, N, C)
            for r in res.results
        ],
        axis=0,
    )
    return full, res


def kernel(**inputs) -> np.ndarray:
    return run(inputs, trace=False)[0]
